# revision 1
# baseline (speedup 1.0000x reference)
"""Dense linear attention (elu+1 feature map) Trainium2 Bass kernel.

Problem: B=8, T=4096, D=1024, H=64.
  q = phi(x @ W_q), k = phi(x @ W_k), v = x @ W_v          (phi = elu+1)
  S_t = S_{t-1} + k_t v_t^T ; z_t = z_{t-1} + k_t          (S[i,j] = sum k_i v_j)
  o_t = (S_t q_t) / max(z_t . q_t, 1e-6)                    (o_i = sum_j S[i,j] q_j)
  y = o @ W_o

Note the reference einsum contracts q against the *v* index of S in the
numerator (o_t = sum_{s<=t} k_s (v_s . q_t)) while the denominator uses
z = sum k. The chunked form (C=128) per chunk:
  Av[s,t] = v_s . q_t   (masked s<=t)      -> numerator intra
  Ak[s,t] = k_s . q_t   (masked s<=t)      -> denominator intra
  O^T[i,t] = K^T Av_m + S_vk^T q^T         (S_vk[j,i] = sum v_j k_i)
  dcol[t]  = colsum(Ak_m) + q_t . z
Data-parallel over batch: one batch element per NeuronCore (8 cores).
All matmuls bf16 with fp32 PSUM accumulation; x transposed on PE in fp32.
"""

import numpy as np

import concourse.bass as bass
import concourse.mybir as mybir
import concourse.tile as tile
from concourse import bacc
from concourse.bass_utils import run_bass_kernel_spmd
from concourse.masks import make_identity, make_upper_triangular

F32 = mybir.dt.float32
BF16 = mybir.dt.bfloat16
AF = mybir.ActivationFunctionType

B, T, D, H = 8, 4096, 1024, 64
C = 128                 # chunk (recurrence step block)
BLK = 512               # projection block: 4 chunks
N_BLK = T // BLK        # 8
N_CH = BLK // C         # 4 chunks per block
DJ = D // 128           # 8 contraction sub-tiles


def build_nc(reps=1):
    nc = bacc.Bacc("TRN2", target_bir_lowering=False, debug=False)

    x_d = nc.dram_tensor("x", [T, D], F32, kind="ExternalInput")
    wq_d = nc.dram_tensor("wq", [D, H], F32, kind="ExternalInput")
    wk_d = nc.dram_tensor("wk", [D, H], F32, kind="ExternalInput")
    wv_d = nc.dram_tensor("wv", [D, H], F32, kind="ExternalInput")
    wo_d = nc.dram_tensor("wo", [H, D], F32, kind="ExternalInput")
    y_d = nc.dram_tensor("y", [T, D], F32, kind="ExternalOutput")

    with tile.TileContext(nc) as tc:
        with (
            tc.tile_pool(name="const", bufs=1) as const,
            tc.tile_pool(name="xin", bufs=4) as xin,
            tc.tile_pool(name="xtbf", bufs=2) as xtbf,
            tc.tile_pool(name="phi", bufs=2) as phip,
            tc.tile_pool(name="chunk", bufs=6) as chp,
            tc.tile_pool(name="state", bufs=4) as stp,
            tc.tile_pool(name="yout", bufs=6) as yp,
            tc.tile_pool(name="ps_xt", bufs=1, space="PSUM") as ps_xt,
            tc.tile_pool(name="ps_qk", bufs=1, space="PSUM") as ps_qk,
            tc.tile_pool(name="ps_y", bufs=2, space="PSUM") as ps_y,
            tc.tile_pool(name="ps_sm", bufs=4, space="PSUM") as ps_sm,
        ):
            # ---- constants / weights ----
            ident_f = const.tile([128, 128], F32, tag="identf")
            make_identity(nc, ident_f[:])
            ident_b = const.tile([128, 128], BF16, tag="identb")
            make_identity(nc, ident_b[:])
            # mask[s,t] = 1.0 where s <= t (upper triangular incl diagonal)
            mask = const.tile([128, 128], F32, tag="mask")
            make_upper_triangular(nc, mask[:], val=1.0, diag=True)
            ones_b = const.tile([128, 1], BF16, tag="ones")
            nc.vector.memset(ones_b[:], 1.0)

            # weight staging (fp32) and bf16 conversion
            wq_st = const.tile([128, DJ, H], F32, tag="wqst")
            wk_st = const.tile([128, DJ, H], F32, tag="wkst")
            wv_st = const.tile([128, DJ, H], F32, tag="wvst")
            wo_st = const.tile([H, D], F32, tag="wost")
            nc.sync.dma_start(wq_st[:], wq_d.rearrange("(j p) h -> p j h", p=128))
            nc.sync.dma_start(wk_st[:], wk_d.rearrange("(j p) h -> p j h", p=128))
            nc.sync.dma_start(wv_st[:], wv_d.rearrange("(j p) h -> p j h", p=128))
            nc.sync.dma_start(wo_st[:], wo_d[:])

            wqk_b = const.tile([128, DJ, 2 * H], BF16, tag="wqkb")
            wv_b = const.tile([128, DJ, H], BF16, tag="wvb")
            wo_b = const.tile([H, D], BF16, tag="wob")
            nc.vector.tensor_copy(wqk_b[:, :, 0:H], wq_st[:])
            nc.vector.tensor_copy(wqk_b[:, :, H : 2 * H], wk_st[:])
            nc.vector.tensor_copy(wv_b[:], wv_st[:])
            nc.vector.tensor_copy(wo_b[:], wo_st[:])

            def emit_front(b):
                    t0 = b * BLK
                    # ---- load x block, transpose on PE, cast to bf16 ----
                    xt_b = xtbf.tile([128, DJ, N_CH, 128], BF16, tag="xt")
                    for ci in range(N_CH):
                        x_sb = xin.tile([128, D], F32, tag="x")
                        nc.sync.dma_start(
                            x_sb[:], x_d[t0 + ci * C : t0 + (ci + 1) * C, :]
                        )
                        for half in range(2):
                            xt_ps = ps_xt.tile([128, 4, 128], F32, tag="xtp")
                            for jj in range(4):
                                j = half * 4 + jj
                                nc.tensor.transpose(
                                    xt_ps[:, jj, :],
                                    x_sb[:, j * 128 : (j + 1) * 128],
                                    ident_f[:],
                                )
                            nc.scalar.copy(
                                xt_b[:, half * 4 : half * 4 + 4, ci, :], xt_ps[:]
                            )

                    # ---- q/k projection: [Wq|Wk]^T x^T -> [128, BLK] psum ----
                    qk_ps = ps_qk.tile([128, BLK], F32, tag="qk")
                    for j in range(DJ):
                        nc.tensor.matmul(
                            qk_ps[:],
                            wqk_b[:, j, :],
                            xt_b[:, j, :, :],
                            start=(j == 0),
                            stop=(j == DJ - 1),
                        )

                    # ---- phi = relu(x) + exp(min(x,0)), split into q/k ----
                    t1 = phip.tile([128, BLK], F32, tag="t1")
                    nc.scalar.activation(t1[:], qk_ps[:], AF.Relu, scale=-1.0)
                    t2 = phip.tile([128, BLK], F32, tag="t2")
                    nc.scalar.activation(t2[:], t1[:], AF.Exp, scale=-1.0)
                    t3 = phip.tile([128, BLK], F32, tag="t3")
                    nc.vector.tensor_scalar_max(t3[:], qk_ps[:], 0.0)
                    q_phi = phip.tile([H, BLK], BF16, tag="qphi")
                    k_phi = phip.tile([H, BLK], BF16, tag="kphi")
                    nc.vector.tensor_add(q_phi[:], t2[0:H, :], t3[0:H, :])
                    nc.vector.tensor_add(k_phi[:], t2[H:128, :], t3[H:128, :])
                    return t0, xt_b, q_phi, k_phi

            def body():
                # rotating state (fp32 accumulator + bf16 copy for matmuls)
                st = {"s_f32": None, "s_bf": None, "c_glob": 0}

                def emit_chunks(front):
                    t0, xt_b, q_phi, k_phi = front
                    s_f32_prev = st["s_f32"]
                    s_bf_prev = st["s_bf"]
                    c_glob = st["c_glob"]
                    # ---- per-chunk recurrence ----
                    for ci in range(N_CH):
                        cs = slice(ci * C, (ci + 1) * C)
                        first = c_glob == 0

                        # K chunk in [s, h] layout via PE transpose of k_phi
                        kt_ps = ps_sm.tile([128, H], BF16, tag="sm")
                        nc.tensor.transpose(
                            kt_ps[:], k_phi[:, cs], ident_b[0:H, 0:H]
                        )
                        k_sb = chp.tile([128, H], BF16, tag="ksb")
                        nc.scalar.copy(k_sb[:], kt_ps[:])

                        # V chunk [s, h] via 8 accumulated matmuls, + transpose
                        v_ps = ps_sm.tile([128, H], F32, tag="sm")
                        for j in range(DJ):
                            nc.tensor.matmul(
                                v_ps[:],
                                xt_b[:, j, ci, :],
                                wv_b[:, j, :],
                                start=(j == 0),
                                stop=(j == DJ - 1),
                            )
                        v_sb = chp.tile([128, H], BF16, tag="vsb")
                        nc.scalar.copy(v_sb[:], v_ps[:])
                        vt_ps = ps_sm.tile([H, 128], BF16, tag="sm")
                        nc.tensor.transpose(vt_ps[:], v_sb[:], ident_b[:])
                        vt_sb = chp.tile([H, 128], BF16, tag="vtsb")
                        nc.scalar.copy(vt_sb[:], vt_ps[:])

                        # Av[s,t] = v_s . q_t (numerator); Ak[s,t] = k_s . q_t
                        av_ps = ps_sm.tile([128, 128], F32, tag="sm")
                        nc.tensor.matmul(
                            av_ps[:], vt_sb[:], q_phi[:, cs], start=True, stop=True
                        )
                        av_m = chp.tile([128, 128], BF16, tag="avm")
                        nc.vector.tensor_mul(av_m[:], av_ps[:], mask[:])

                        ak_ps = ps_sm.tile([128, 128], F32, tag="sm")
                        nc.tensor.matmul(
                            ak_ps[:], k_phi[:, cs], q_phi[:, cs], start=True, stop=True
                        )
                        ak_m = chp.tile([128, 128], BF16, tag="akm")
                        nc.vector.tensor_mul(ak_m[:], ak_ps[:], mask[:])

                        # O^T[i,t] = sum_s k_s[i] Av_m[s,t] + sum_j S_vk[j,i] q_t[j]
                        ot_ps = ps_sm.tile([H, 128], F32, tag="sm")
                        nc.tensor.matmul(
                            ot_ps[:], k_sb[:], av_m[:], start=True, stop=first
                        )
                        if not first:
                            nc.tensor.matmul(
                                ot_ps[:],
                                s_bf_prev[:, 0:H],
                                q_phi[:, cs],
                                start=False,
                                stop=True,
                            )
                        o_sc = chp.tile([H, 128], BF16, tag="osc")
                        nc.scalar.copy(o_sc[:], ot_ps[:])

                        # denom column: sum_s Ak_m[s,t] + z_prev . q_t
                        d_ps = ps_sm.tile([128, 1], F32, tag="sm")
                        nc.tensor.matmul(
                            d_ps[:], ak_m[:], ones_b[:], start=True, stop=first
                        )
                        if not first:
                            nc.tensor.matmul(
                                d_ps[:],
                                q_phi[:, cs],
                                s_bf_prev[:, H : H + 1],
                                start=False,
                                stop=True,
                            )
                        r_col = chp.tile([128, 1], F32, tag="rcol")
                        nc.vector.tensor_scalar_max(r_col[:], d_ps[:], 1e-6)
                        nc.vector.reciprocal(r_col[:], r_col[:])

                        # state: dS[j,i] = sum_s v_s[j] k_s[i]; dz[i] = sum k_s[i]
                        ds_ps = ps_sm.tile([H, H + 1], F32, tag="sm")
                        nc.tensor.matmul(
                            ds_ps[:, 0:H], v_sb[:], k_sb[:], start=True, stop=True
                        )
                        nc.tensor.matmul(
                            ds_ps[:, H : H + 1],
                            k_sb[:],
                            ones_b[:],
                            start=True,
                            stop=True,
                        )
                        s_f32 = stp.tile([H, H + 1], F32, tag="sf")
                        if first:
                            nc.vector.tensor_copy(s_f32[:], ds_ps[:])
                        else:
                            nc.vector.tensor_add(s_f32[:], ds_ps[:], s_f32_prev[:])
                        s_bf = stp.tile([H, H + 1], BF16, tag="sb")
                        nc.vector.tensor_copy(s_bf[:], s_f32[:])
                        s_f32_prev, s_bf_prev = s_f32, s_bf

                        # output projection + normalization on eviction
                        for half in range(2):
                            nd = slice(half * 512, (half + 1) * 512)
                            y_ps = ps_y.tile([128, 512], F32, tag="y")
                            nc.tensor.matmul(
                                y_ps[:], o_sc[:], wo_b[:, nd], start=True, stop=True
                            )
                            y_sb = yp.tile([128, 512], F32, tag="ysb")
                            nc.vector.tensor_scalar_mul(
                                y_sb[:], y_ps[:], r_col[:, 0:1]
                            )
                            # SWDGE path: keeps output stores off the sync
                            # ring so they never block upcoming x loads
                            nc.gpsimd.dma_start(
                                y_d[t0 + ci * C : t0 + (ci + 1) * C, nd], y_sb[:]
                            )

                        c_glob += 1

                    st["s_f32"] = s_f32_prev
                    st["s_bf"] = s_bf_prev
                    st["c_glob"] = c_glob

                # software pipeline: emit block b+1's load/transpose/
                # projection/phi ahead of block b's serial chunk chain so
                # the scheduler has independent PE work to fill stalls
                front = emit_front(0)
                for b in range(1, N_BLK):
                    nxt = emit_front(b)
                    emit_chunks(front)
                    front = nxt
                emit_chunks(front)

            if reps == 1:
                body()
            else:
                with tc.For_i(0, reps, 1):
                    body()

    nc.compile()
    return nc


_NC = None


def _get_nc():
    global _NC
    if _NC is None:
        _NC = build_nc()
    return _NC


def kernel(x, W_q, W_k, W_v, W_o):
    nc = _get_nc()
    x = np.ascontiguousarray(x, dtype=np.float32)
    wq = np.ascontiguousarray(W_q, dtype=np.float32)
    wk = np.ascontiguousarray(W_k, dtype=np.float32)
    wv = np.ascontiguousarray(W_v, dtype=np.float32)
    wo = np.ascontiguousarray(W_o, dtype=np.float32)
    in_maps = [
        {"x": x[b], "wq": wq, "wk": wk, "wv": wv, "wo": wo} for b in range(B)
    ]
    res = run_bass_kernel_spmd(nc, in_maps, core_ids=list(range(B)))
    return np.stack([res.results[b]["y"] for b in range(B)], axis=0)



# revision 10
# speedup vs baseline: 3.4082x; 3.4082x over previous
"""Dense linear attention (elu+1 feature map) Trainium2 Bass kernel, v2.

Problem: B=8, T=4096, D=1024, H=64.
  q = phi(x @ W_q), k = phi(x @ W_k), v = x @ W_v          (phi = elu+1)
  S_t = S_{t-1} + k_t v_t^T ; z_t = z_{t-1} + k_t          (S[i,j] = sum k_i v_j)
  o_t = (S_t q_t) / max(z_t . q_t, 1e-6)                    (o_i = sum_j S[i,j] q_j)
  y = o @ W_o

The reference einsum contracts q against the *v* index of S in the numerator
(o_t = sum_{s<=t} k_s (v_s . q_t)); the denominator uses z = sum k.
Chunked form (C=128) per chunk:
  Av[s,t] = v_s . q_t  (masked s<=t)  ;  Ak[s,t] = k_s . q_t (masked)
  O^T[i,t] = K_c^T Av_m + S_prev^T-ish q    ; d[t] = colsum(Ak_m) + q_t . z_prev
Data-parallel over batch: one batch element per NeuronCore (8 cores).

v2 changes vs baseline:
  - x is transposed + cast to bf16 on the HOST (free); no on-chip fp32 PE
    transposes of x, DMA traffic halves (8.4 MB in vs 16.8 MB).
  - y output is bf16 (host upcasts); output DMA traffic halves.
  - v projected directly in [h, t] layout (W_v stationary); k/v chunk
    transposes batched into one PSUM bank with a single eviction per block.
  - All small per-chunk matmuls (Av/Ak/dS/dz/d1/d2/O) packed into ONE psum
    bank per chunk; only the O group is multi-matmul (accumulating), all
    others are single-instruction groups, so bank-clear (has_written)
    hazards cannot corrupt accumulation.
  - State kept in fp32 on-chip, with a LAGGED bf16 copy used by the matmuls:
    ssum(c) = cast(S(c-2)) + dS_bf(c-1), so no tight PE->DVE->PE serial
    chain per chunk.
  - phi/evictions spread across ACT / DVE / GPSIMD by hand.
"""

import numpy as np
import ml_dtypes

import concourse.bass as bass
import concourse.mybir as mybir
import concourse.tile as tile
from concourse import bacc
from concourse.bass_utils import run_bass_kernel_spmd
from concourse.masks import make_identity, make_upper_triangular

F32 = mybir.dt.float32
BF16 = mybir.dt.bfloat16
AF = mybir.ActivationFunctionType

B, T, D, H = 8, 4096, 1024, 64
C = 128                 # chunk (recurrence step block)
BLK = 512               # projection block: 4 chunks
N_BLK = T // BLK        # 8
N_CH = BLK // C         # 4 chunks per block
DJ = D // 128           # 8 contraction sub-tiles

# packed small-psum bank column layout (fp32 words)
_AV = slice(0, 128)
_AK = slice(128, 256)
_DS = slice(256, 320)        # [0:64] partitions
_DZ = slice(320, 321)        # [0:64] partitions
_DSZ = slice(256, 321)       # dS and dz together
_O = slice(321, 449)         # [0:64] partitions
SMW = 449


def build_nc(reps=1):
    nc = bacc.Bacc("TRN2", target_bir_lowering=False, debug=False)

    xt_d = nc.dram_tensor("xt", [D, T], BF16, kind="ExternalInput")
    wqk_d = nc.dram_tensor("wqk", [D, 2 * H], BF16, kind="ExternalInput")
    wv_d = nc.dram_tensor("wv", [D, H], BF16, kind="ExternalInput")
    wo_d = nc.dram_tensor("wo", [H, D], BF16, kind="ExternalInput")
    y_d = nc.dram_tensor("y", [T, D], BF16, kind="ExternalOutput")

    with tile.TileContext(nc) as tc:
        with (
            tc.tile_pool(name="const", bufs=1) as const,
            tc.tile_pool(name="xin", bufs=3) as xin,
            tc.tile_pool(name="phi", bufs=2) as phip,
            tc.tile_pool(name="chunk", bufs=3) as chp,
            tc.tile_pool(name="state", bufs=3) as stp,
            tc.tile_pool(name="yout", bufs=3) as yp,
            tc.tile_pool(name="ps_qk", bufs=1, space="PSUM") as ps_qk,
            tc.tile_pool(name="ps_v", bufs=1, space="PSUM") as ps_v,
            tc.tile_pool(name="ps_tr", bufs=1, space="PSUM") as ps_tr,
            tc.tile_pool(name="ps_sm", bufs=2, space="PSUM") as ps_sm,
            tc.tile_pool(name="ps_d", bufs=1, space="PSUM") as ps_d,
            tc.tile_pool(name="ps_y", bufs=2, space="PSUM") as ps_y,
        ):
            # ---- constants / weights ----
            ident_b = const.tile([128, 128], BF16, tag="identb")
            make_identity(nc, ident_b[:])
            # mask[s,t] = 1.0 where s <= t, duplicated for the packed Av|Ak mul
            mask2 = const.tile([128, 256], F32, tag="mask2")
            make_upper_triangular(nc, mask2[:, 0:128], val=1.0, diag=True)
            make_upper_triangular(nc, mask2[:, 128:256], val=1.0, diag=True)
            ones_b = const.tile([128, 1], BF16, tag="ones")
            nc.vector.memset(ones_b[:], 1.0)

            wqk_b = const.tile([128, DJ, 2 * H], BF16, tag="wqkb")
            wv_b = const.tile([128, DJ, H], BF16, tag="wvb")
            wo_b = const.tile([H, D], BF16, tag="wob")
            nc.sync.dma_start(wqk_b[:], wqk_d.rearrange("(j p) h -> p j h", p=128))
            nc.sync.dma_start(wv_b[:], wv_d.rearrange("(j p) h -> p j h", p=128))
            nc.sync.dma_start(wo_b[:], wo_d[:])

            def emit_front(b):
                t0 = b * BLK
                # ---- load x^T block (bf16, pre-transposed on host) ----
                xt_t = xin.tile([128, DJ, BLK], BF16, tag="xt")
                for j in range(DJ):
                    nc.sync.dma_start(
                        xt_t[:, j, :], xt_d[j * 128 : (j + 1) * 128, t0 : t0 + BLK]
                    )

                # ---- q/k projection: [Wq|Wk]^T x^T -> [128, BLK] psum ----
                qk_ps = ps_qk.tile([128, BLK], F32, tag="qk")
                for j in range(DJ):
                    nc.tensor.matmul(
                        qk_ps[:],
                        wqk_b[:, j, :],
                        xt_t[:, j, :],
                        start=(j == 0),
                        stop=(j == DJ - 1),
                    )

                # ---- phi = exp(min(x,0)) + max(x,0) ----
                t1 = phip.tile([128, BLK], F32, tag="t1")
                nc.scalar.activation(t1[:], qk_ps[:], AF.Relu, scale=-1.0)
                t2 = phip.tile([128, BLK], F32, tag="t2")
                nc.scalar.activation(t2[:], t1[:], AF.Exp, scale=-1.0)
                t3 = phip.tile([128, BLK], F32, tag="t3")
                nc.vector.tensor_scalar_max(t3[:], qk_ps[:], 0.0)
                q_phi = phip.tile([H, BLK], BF16, tag="qphi")
                nc.gpsimd.tensor_add(q_phi[:], t2[0:H, :], t3[0:H, :])
                k_phi = phip.tile([H, BLK], BF16, tag="kphi")
                nc.gpsimd.tensor_add(k_phi[:], t2[H:128, :], t3[H:128, :])

                # ---- v projection directly in [h, t]: Wv^T x^T ----
                v_ps = ps_v.tile([H, BLK], F32, tag="v")
                for j in range(DJ):
                    nc.tensor.matmul(
                        v_ps[:],
                        wv_b[:, j, :],
                        xt_t[:, j, :],
                        start=(j == 0),
                        stop=(j == DJ - 1),
                    )
                vt_sb = phip.tile([H, BLK], BF16, tag="vtsb")
                nc.scalar.copy(vt_sb[:], v_ps[:])

                # ---- per-chunk k,v transposes to [s, h], batched in 1 bank ----
                tr_ps = ps_tr.tile([128, BLK], BF16, tag="tr")
                for ci in range(N_CH):
                    cs = slice(ci * C, (ci + 1) * C)
                    nc.tensor.transpose(
                        tr_ps[:, ci * H : (ci + 1) * H],
                        k_phi[:, cs],
                        ident_b[0:H, 0:H],
                    )
                for ci in range(N_CH):
                    cs = slice(ci * C, (ci + 1) * C)
                    nc.tensor.transpose(
                        tr_ps[:, 256 + ci * H : 256 + (ci + 1) * H],
                        vt_sb[:, cs],
                        ident_b[0:H, 0:H],
                    )
                kv_sb = phip.tile([128, BLK], BF16, tag="kvsb")
                nc.scalar.copy(kv_sb[:], tr_ps[:])
                return t0, q_phi, k_phi, vt_sb, kv_sb

            def body():
                st = {
                    "c": 0,          # global chunk counter
                    "s_f32": None,   # fp32 running state [64, 65] (S | z)
                    "t_bf": None,    # bf16 cast of s_f32, lagged
                    "dS_bf": None,   # bf16 copy of last chunk's dS
                    "ssum": None,    # bf16 state used by chunk c's matmuls
                }

                def emit_chunks(front):
                    t0, q_phi, k_phi, vt_sb, kv_sb = front
                    for ci in range(N_CH):
                        c = st["c"]
                        first = c == 0
                        cs = slice(ci * C, (ci + 1) * C)
                        q_c = q_phi[:, cs]
                        k_c = k_phi[:, cs]
                        k_sb = kv_sb[:, ci * H : (ci + 1) * H]
                        v_sb = kv_sb[:, 256 + ci * H : 256 + (ci + 1) * H]
                        ssum = st["ssum"]

                        sm = ps_sm.tile([128, SMW], F32, tag="sm")

                        # Av[s,t] = v_s.q_t ; Ak[s,t] = k_s.q_t  (adjacent cols)
                        nc.tensor.matmul(
                            sm[:, _AV], vt_sb[:, cs], q_c, start=True, stop=True
                        )
                        nc.tensor.matmul(
                            sm[:, _AK], k_c, q_c, start=True, stop=True
                        )
                        avak_m = chp.tile([128, 256], BF16, tag="avakm")
                        nc.vector.tensor_mul(avak_m[:], sm[:, 0:256], mask2[:])

                        # dS[j,i] = sum_s v[s,j] k[s,i] ; dz[i] = sum_s k[s,i]
                        nc.tensor.matmul(
                            sm[0:H, _DS], v_sb, k_sb, start=True, stop=True
                        )
                        nc.tensor.matmul(
                            sm[0:H, _DZ], k_sb, ones_b[:], start=True, stop=True
                        )
                        dS_bf = stp.tile([H, H + 1], BF16, tag="dsbf")
                        nc.scalar.copy(dS_bf[:], sm[0:H, _DSZ])
                        s_f32 = stp.tile([H, H + 1], F32, tag="sf")
                        if first:
                            nc.vector.tensor_copy(s_f32[:], sm[0:H, _DSZ])
                        else:
                            nc.vector.tensor_add(
                                s_f32[:], sm[0:H, _DSZ], st["s_f32"]
                            )
                        t_bf = stp.tile([H, H + 1], BF16, tag="tbf")
                        nc.gpsimd.tensor_copy(t_bf[:], s_f32[:])
                        # state for chunk c+1: ssum(c+1) = cast(S(c-1)) + dS(c)
                        if first:
                            ssum_next = dS_bf
                        else:
                            ssum_next = stp.tile([H, H + 1], BF16, tag="ssum")
                            nc.gpsimd.tensor_add(
                                ssum_next[:], st["t_bf"][:], dS_bf[:]
                            )

                        # denominator: own psum bank (multi-matmul group)
                        d_ps = ps_d.tile([128, 1], F32, tag="d")
                        nc.tensor.matmul(
                            d_ps[:], avak_m[:, 128:256], ones_b[:],
                            start=True, stop=first,
                        )
                        if not first:
                            nc.tensor.matmul(
                                d_ps[:], q_c, ssum[:, H : H + 1],
                                start=False, stop=True,
                            )

                        # O^T[i,t] = sum_s k[s,i] Av_m[s,t] (+ S_prev q)
                        nc.tensor.matmul(
                            sm[0:H, _O], k_sb, avak_m[:, 0:128],
                            start=True, stop=first,
                        )
                        if not first:
                            nc.tensor.matmul(
                                sm[0:H, _O], ssum[:, 0:H], q_c,
                                start=False, stop=True,
                            )
                        o_sc = chp.tile([H, C], BF16, tag="osc")
                        nc.vector.tensor_copy(o_sc[:], sm[0:H, _O])

                        # r = 1 / max(d, 1e-6)
                        r_col = chp.tile([128, 1], F32, tag="rcol")
                        nc.vector.tensor_scalar_max(r_col[:], d_ps[:], 1e-6)
                        nc.vector.reciprocal(r_col[:], r_col[:])

                        # y = (o @ Wo) * r ; evict as bf16, one DMA per chunk
                        y_sb = yp.tile([128, D], BF16, tag="ysb")
                        for half in range(2):
                            nd = slice(half * 512, (half + 1) * 512)
                            y_ps = ps_y.tile([128, 512], F32, tag="y")
                            nc.tensor.matmul(
                                y_ps[:], o_sc[:], wo_b[:, nd], start=True, stop=True
                            )
                            nc.scalar.activation(
                                y_sb[:, nd], y_ps[:], AF.Copy, scale=r_col[:, 0:1]
                            )
                        nc.scalar.dma_start(
                            y_d[t0 + ci * C : t0 + (ci + 1) * C, :], y_sb[:]
                        )

                        st["c"] = c + 1
                        st["s_f32"] = s_f32
                        st["t_bf"] = t_bf
                        st["dS_bf"] = dS_bf
                        st["ssum"] = ssum_next

                # software pipeline: next block's front is emitted ahead of
                # this block's chunk chain so PE always has independent work
                front = emit_front(0)
                for b in range(1, N_BLK):
                    nxt = emit_front(b)
                    emit_chunks(front)
                    front = nxt
                emit_chunks(front)

            if reps == 1:
                body()
            else:
                with tc.For_i(
                    0, reps, 1,
                    hint_engines=(
                        mybir.EngineType.PE,
                        mybir.EngineType.Activation,
                        mybir.EngineType.DVE,
                    ),
                ):
                    body()

    nc.compile()
    return nc


_NC = None


def _get_nc():
    global _NC
    if _NC is None:
        _NC = build_nc()
    return _NC


def make_in_maps(x, W_q, W_k, W_v, W_o):
    bf = ml_dtypes.bfloat16
    wqk = np.ascontiguousarray(
        np.concatenate([W_q, W_k], axis=1), dtype=np.float32
    ).astype(bf)
    wv = np.ascontiguousarray(W_v, dtype=np.float32).astype(bf)
    wo = np.ascontiguousarray(W_o, dtype=np.float32).astype(bf)
    maps = []
    for b in range(B):
        xt = np.ascontiguousarray(np.asarray(x[b], dtype=np.float32).T).astype(bf)
        maps.append({"xt": xt, "wqk": wqk, "wv": wv, "wo": wo})
    return maps


def kernel(x, W_q, W_k, W_v, W_o):
    nc = _get_nc()
    in_maps = make_in_maps(x, W_q, W_k, W_v, W_o)
    res = run_bass_kernel_spmd(nc, in_maps, core_ids=list(range(B)))
    return np.stack(
        [res.results[b]["y"].astype(np.float32) for b in range(B)], axis=0
    )
